# revision 13
# baseline (speedup 1.0000x reference)
"""3-layer GATv2 (heads=1, eval) on 8 Trainium2 NeuronCores — Bass/Tile.

kernel(**inputs) takes the FULL inputs (x [100000,128] f32, Wl/Wr [3,128,128],
att [3,128], b [3,128], edge_index [2,1600000] int64) and returns the FULL
[100000, 128] float32 output.

Strategy (graph/data parallel, node-partitioned):
  * core c owns dst nodes [c*12500, (c+1)*12500); edges grouped on the host by
    (dst block of 128 nodes, src bucket of 25000 rows) with a uniform
    per-(block,bucket) slot budget B1 (multiple of 128). Slot order within a
    block is j-major: s = j*NBUCK + k (j = 128-slot sub-column, k = bucket) so
    one 512-slot chunk = one j across all 4 buckets.
  * per layer, XL = h@Wl for all N rows (bf16) lives in DRAM (built locally,
    AllGather'd); XR = h@Wr for local rows lives in a persistent SBUF tile.
  * ONLY xl is fetched per-edge, with SWDGE dma_gather (int16 idx, 256B rows)
    spread over 4 SWDGE queues (one per bucket). xr per edge is expanded on
    the PE: one-hot OHsT[n, e] = [dloc_e == n] (one batched DVE is_equal from
    a host-replicated int8 dloc table) times the 128-row XR block.
  * scores: v = xl[src]+xr[dst]; z = LeakyReLU(v) on the scalar engine;
    e = sum_d z*att (DVE mult+reduce); w = exp(e) (no segment-max: |e| < ~30).
  * aggregation per block on PE: lhsT = OH[e, n] (batched DVE is_equal build),
    rhs = [w*xl | w] (129 cols) accumulated over the 20 slot columns in PSUM;
    col 128 gives the softmax denominator. out = nums/den + bias (+relu).
  * next layer's XL rows + XR block produced in the same block pass
    (PE transpose + two 128x128 matmuls); only the AllGather separates layers.
"""

import os
from contextlib import ExitStack

import numpy as np
import ml_dtypes

import concourse.bacc as bacc
import concourse.mybir as mybir
import concourse.tile as tile
from concourse._compat import cdiv
from concourse.masks import make_identity
from concourse.bass_utils import run_bass_kernel_spmd

F32 = mybir.dt.float32
BF16 = mybir.dt.bfloat16
I16 = mybir.dt.int16
I8 = mybir.dt.int8
AX = mybir.AxisListType
OP = mybir.AluOpType
ACTF = mybir.ActivationFunctionType

D = 128
P = 128


class Cfg:
    def __init__(self, N, cores, bucket, b1, sb):
        assert N % cores == 0
        self.N, self.CORES = N, cores
        self.NPC = N // cores
        self.NBLK = cdiv(self.NPC, P)
        self.LASTW = self.NPC - (self.NBLK - 1) * P
        self.BUCKET = bucket
        self.NBUCK = cdiv(N, bucket)
        assert b1 % P == 0
        self.B1 = b1
        self.JJ = b1 // P
        self.S = self.NBUCK * self.JJ
        self.SB = sb
        self.NSB = cdiv(self.NBLK, sb)
        self.IDXCOLS_TOT = sum(
            self.sbn(g) * self.B1 // 16 * self.NBUCK for g in range(self.NSB)
        )

    def sbn(self, g):
        return min(self.SB, self.NBLK - g * self.SB)


def _wrap16(v):
    L = v.size
    assert L % 16 == 0
    w = v.reshape(L // 16, 16).T.astype(np.int16)
    return np.tile(w, (8, 1))


def host_prep(cfg, edge_index):
    src = np.asarray(edge_index[0], dtype=np.int64)
    dst = np.asarray(edge_index[1], dtype=np.int64)
    cores = []
    for c in range(cfg.CORES):
        base = c * cfg.NPC
        m = (dst >= base) & (dst < base + cfg.NPC)
        es, ed = src[m], dst[m] - base
        blk = ed // P
        buck = es // cfg.BUCKET
        order = np.lexsort((es, buck, blk))
        es, ed, blk, buck = es[order], ed[order], blk[order], buck[order]
        key = blk * cfg.NBUCK + buck
        bounds = np.searchsorted(key, np.arange(cfg.NBLK * cfg.NBUCK + 1))
        cnt = np.diff(bounds).reshape(cfg.NBLK, cfg.NBUCK)
        if cnt.max() > cfg.B1:
            raise ValueError(f"bucket overflow: {cnt.max()} > {cfg.B1}")
        xl_slots = np.zeros((cfg.NBLK, cfg.NBUCK, cfg.B1), np.int64)
        dl_slots = np.full((cfg.NBLK, cfg.NBUCK, cfg.B1), -1.0, np.float32)
        for b in range(cfg.NBLK):
            for k in range(cfg.NBUCK):
                i0, i1 = bounds[b * cfg.NBUCK + k], bounds[b * cfg.NBUCK + k + 1]
                n = i1 - i0
                xl_slots[b, k, :n] = es[i0:i1] - k * cfg.BUCKET
                dl_slots[b, k, :n] = (ed[i0:i1] - b * P).astype(np.float32)
        xl_cols = []
        for g in range(cfg.NSB):
            sbn = cfg.sbn(g)
            for k in range(cfg.NBUCK):
                xl_cols.append(
                    _wrap16(xl_slots[g * cfg.SB : g * cfg.SB + sbn, k, :].reshape(-1))
                )
        xl_idx = np.concatenate(xl_cols, axis=1)
        # j-major slot order: s = j*NBUCK + k
        dl = (
            dl_slots.reshape(cfg.NBLK, cfg.NBUCK, cfg.JJ, P)
            .transpose(0, 2, 1, 3)
            .reshape(cfg.NBLK, cfg.S, P)
        )
        dloc = np.ascontiguousarray(dl.transpose(2, 0, 1).reshape(P, cfg.NBLK * cfg.S))
        dlb_row = dl.reshape(cfg.NBLK * cfg.S * P).astype(np.int8)
        dlb = np.ascontiguousarray(
            np.broadcast_to(dlb_row[None, :], (P, dlb_row.size))
        )
        cores.append(dict(xl_idx=xl_idx, dloc=dloc, dlb=dlb))
    return cores


def host_consts(cfg, Wl, Wr, att, b, x):
    Wl = np.asarray(Wl, np.float32).copy()
    Wr = np.asarray(Wr, np.float32).copy()
    att = np.asarray(att, np.float32)
    b = np.asarray(b, np.float32).copy()
    x = np.asarray(x, np.float32)
    # fold |att| + sign-sorted feature permutation into layers 0/1 so the
    # per-edge score is a sign-split column sum; layer 2 stays plain.
    perms, scales, Ks = [], [], []
    for l in range(2):
        a = att[l]
        pos = np.where(a >= 0)[0]
        neg = np.where(a < 0)[0]
        perm = np.concatenate([pos, neg])
        s = np.maximum(np.abs(a[perm]), 1e-6)
        perms.append(perm); scales.append(s); Ks.append(len(pos))
    Wl_e = [None] * 3
    Wr_e = [None] * 3
    b_e = [None] * 3
    Wl_e[0] = Wl[0][:, perms[0]] * scales[0][None, :]
    Wr_e[0] = Wr[0][:, perms[0]] * scales[0][None, :]
    b_e[0] = b[0][perms[0]] * scales[0]
    Wl_e[1] = ((1.0 / scales[0])[:, None] * Wl[1][perms[0], :])[:, perms[1]] \
        * scales[1][None, :]
    Wr_e[1] = ((1.0 / scales[0])[:, None] * Wr[1][perms[0], :])[:, perms[1]] \
        * scales[1][None, :]
    b_e[1] = b[1][perms[1]] * scales[1]
    Wl_e[2] = (1.0 / scales[1])[:, None] * Wl[2][perms[1], :]
    Wr_e[2] = (1.0 / scales[1])[:, None] * Wr[2][perms[1], :]
    b_e[2] = b[2]
    wl_all = np.stack(Wl_e).reshape(3 * D, D).astype(ml_dtypes.bfloat16)
    wr_all = np.stack(Wr_e).reshape(3 * D, D).astype(ml_dtypes.bfloat16)
    att_mat = np.concatenate(
        [np.tile(att[l][None, :], (P, 1)) for l in range(3)], 0
    ).astype(ml_dtypes.bfloat16)
    bias_mat = np.concatenate(
        [np.tile(b_e[l][None, :], (P, 1)) for l in range(3)], 0
    ).astype(np.float32)
    iota = np.tile(np.arange(P, dtype=np.float32)[None, :], (P, 1)).astype(
        ml_dtypes.bfloat16
    )
    iota_col = np.arange(P, dtype=np.float32).reshape(P, 1)
    out = []
    for c in range(cfg.CORES):
        xT = np.ascontiguousarray(x[c * cfg.NPC : (c + 1) * cfg.NPC].T).astype(
            ml_dtypes.bfloat16
        )
        out.append(
            dict(
                xT_loc=xT,
                Wl_all=wl_all,
                Wr_all=wr_all,
                att_mat=att_mat,
                bias_mat=bias_mat,
                iota_mat=iota,
                iota_col=iota_col,
            )
        )
    return out


def build_program(cfg, Ks):
    nc = bacc.Bacc(
        "TRN2", target_bir_lowering=False, debug=False, num_devices=cfg.CORES,
        num_swdge_queues=4,
    )
    NPC, NBLK, NBUCK, B1, S, SB, NSB, JJ = (
        cfg.NPC, cfg.NBLK, cfg.NBUCK, cfg.B1, cfg.S, cfg.SB, cfg.NSB, cfg.JJ,
    )

    xT_loc = nc.dram_tensor("xT_loc", [P, NPC], BF16, kind="ExternalInput")
    Wl_all = nc.dram_tensor("Wl_all", [3 * D, D], BF16, kind="ExternalInput")
    Wr_all = nc.dram_tensor("Wr_all", [3 * D, D], BF16, kind="ExternalInput")
    att_mat = nc.dram_tensor("att_mat", [3 * P, D], BF16, kind="ExternalInput")
    bias_mat = nc.dram_tensor("bias_mat", [3 * P, D], F32, kind="ExternalInput")
    iota_mat = nc.dram_tensor("iota_mat", [P, P], BF16, kind="ExternalInput")
    iota_col = nc.dram_tensor("iota_col", [P, 1], F32, kind="ExternalInput")
    xl_idx = nc.dram_tensor("xl_idx", [P, cfg.IDXCOLS_TOT], I16, kind="ExternalInput")
    dloc = nc.dram_tensor("dloc", [P, NBLK * S], F32, kind="ExternalInput")
    dlb_d = nc.dram_tensor("dlb", [P, NBLK * S * P], I8, kind="ExternalInput")
    out_loc = nc.dram_tensor("out_loc", [NPC, D], F32, kind="ExternalOutput")

    XLb = [nc.dram_tensor(f"XLb{l}", [NPC, D], BF16) for l in range(3)]
    XLf = [nc.dram_tensor(f"XLf{l}", [cfg.N, D], BF16) for l in range(3)]

    with tile.TileContext(nc) as tc, ExitStack() as ctx:
        consts = ctx.enter_context(tc.tile_pool(name="consts", bufs=1))
        gpool = ctx.enter_context(tc.tile_pool(name="gath", bufs=2))
        wrk = ctx.enter_context(tc.tile_pool(name="wrk", bufs=2))
        small = ctx.enter_context(tc.tile_pool(name="small", bufs=3))
        idxp = ctx.enter_context(tc.tile_pool(name="idx", bufs=2))
        dlbp = ctx.enter_context(tc.tile_pool(name="dlb", bufs=2))
        psX = ctx.enter_context(tc.tile_pool(name="psX", bufs=2, space="PSUM"))
        psA = ctx.enter_context(tc.tile_pool(name="psA", bufs=2, space="PSUM"))
        psF = ctx.enter_context(tc.tile_pool(name="psF", bufs=2, space="PSUM"))
        psT = ctx.enter_context(tc.tile_pool(name="psT", bufs=2, space="PSUM"))

        iota_t = consts.tile([P, P], BF16, tag="iota")
        nc.sync.dma_start(iota_t[:], iota_mat[:, :])
        iotac = consts.tile([P, 1], F32, tag="iotac")
        nc.sync.dma_start(iotac[:], iota_col[:, :])
        alpha_t = consts.tile([P, 1], F32, tag="alpha")
        nc.vector.memset(alpha_t[:], 0.2)
        ident_t = consts.tile([P, P], BF16, tag="ident")
        make_identity(nc, ident_t[:])
        XR_sb = consts.tile([P, NBLK * D], BF16, tag="xrsb")
        wl_t, wr_t, at_t, bi_t = [], [], [], []
        for l in range(3):
            w1 = consts.tile([P, D], BF16, tag=f"wl{l}")
            nc.sync.dma_start(w1[:], Wl_all[l * D : (l + 1) * D, :])
            w2 = consts.tile([P, D], BF16, tag=f"wr{l}")
            nc.sync.dma_start(w2[:], Wr_all[l * D : (l + 1) * D, :])
            a1 = consts.tile([P, D], BF16, tag=f"att{l}")
            nc.sync.dma_start(a1[:], att_mat[l * P : (l + 1) * P, :])
            b1t = consts.tile([P, D], F32, tag=f"bias{l}")
            nc.sync.dma_start(b1t[:], bias_mat[l * P : (l + 1) * P, :])
            wl_t.append(w1); wr_t.append(w2); at_t.append(a1); bi_t.append(b1t)

        for cblk in range(NBLK):
            cw = P if cblk < NBLK - 1 else cfg.LASTW
            xTs = wrk.tile([P, P], BF16, tag="xTs")
            if cw < P:
                nc.vector.memset(xTs[:, cw:], 0.0)
            nc.sync.dma_start(xTs[:, :cw], xT_loc[:, cblk * P : cblk * P + cw])
            pxl = psF.tile([P, D], F32, tag="fin")
            nc.tensor.matmul(pxl[:], xTs[:], wl_t[0][:], start=True, stop=True)
            sxl = small.tile([P, D], BF16, tag="sxl")
            nc.scalar.activation(sxl[:cw, :], pxl[:cw, :], ACTF.Copy)
            nc.sync.dma_start(XLb[0][cblk * P : cblk * P + cw, :], sxl[:cw, :])
            pxr = psF.tile([P, D], F32, tag="fin")
            nc.tensor.matmul(pxr[:], xTs[:], wr_t[0][:], start=True, stop=True)
            nc.scalar.activation(
                XR_sb[:, cblk * D : cblk * D + D], pxr[:], ACTF.Copy
            )
        nc.gpsimd.collective_compute(
            "AllGather", OP.bypass,
            replica_groups=[list(range(cfg.CORES))],
            ins=[XLb[0].ap().opt()], outs=[XLf[0].ap().opt()],
        )

        for l in range(3):
            goff = 0
            for g in range(NSB):
                sbn = cfg.sbn(g)
                gcols = sbn * B1 // 16
                ixl = idxp.tile([P, NBUCK * gcols], I16, tag="ixl")
                nc.sync.dma_start(ixl[:], xl_idx[:, goff : goff + NBUCK * gcols])
                dlt = small.tile([P, sbn * S], F32, tag="dlt")
                nc.sync.dma_start(
                    dlt[:], dloc[:, g * SB * S : g * SB * S + sbn * S]
                )
                dlb = dlbp.tile([P, sbn * S * P], I8, tag="dlb")
                nc.sync.dma_start(
                    dlb[:],
                    dlb_d[:, g * SB * S * P : g * SB * S * P + sbn * S * P],
                )
                xlg = gpool.tile([P, NBUCK * sbn * B1], BF16, tag="xlg")
                ni = sbn * B1
                for k in range(NBUCK):
                    kb = k * cfg.BUCKET
                    ke = min(kb + cfg.BUCKET, cfg.N)
                    nc.gpsimd.dma_gather(
                        xlg[:, k * ni : (k + 1) * ni].rearrange("p (m x) -> p m x", x=D),
                        XLf[l][kb:ke, :],
                        ixl[:, k * gcols : (k + 1) * gcols],
                        ni, ni, D, single_packet=False,
                        queue_num=k,
                    )
                # [P, NBUCK, sbn, JJ*128] per-bucket rows of this group
                xlg_r = xlg[:].rearrange(
                    "p (k s b1) -> p k s b1", k=NBUCK, s=sbn
                )
                for bl in range(sbn):
                    b = g * SB + bl
                    bw = P if b < NBLK - 1 else cfg.LASTW
                    # one-hot (n-partition) for xr expansion: OHsT[n, s*128+e]
                    ohsT = wrk.tile([P, S * P], BF16, tag="ohsT")
                    nc.gpsimd.tensor_scalar(
                        ohsT[:],
                        dlb[:, bl * S * P : (bl + 1) * S * P],
                        iotac[:, 0:1], None, op0=OP.is_equal,
                    )
                    xrblk = XR_sb[:, b * D : (b + 1) * D]
                    v = wrk.tile([P, NBUCK * B1], BF16, tag="v")
                    for j in range(JJ):
                        pxr = psX.tile([P, NBUCK * P], F32, tag="px")
                        for k in range(NBUCK):
                            s = j * NBUCK + k
                            nc.tensor.matmul(
                                pxr[:, k * P : (k + 1) * P],
                                ohsT[:, s * P : (s + 1) * P],
                                xrblk,
                                start=True, stop=True,
                            )
                        nc.vector.tensor_tensor(
                            v[:].rearrange("p (s x) -> p s x", x=P)[
                                :, j * NBUCK : (j + 1) * NBUCK, :
                            ],
                            xlg_r[:, :, bl, j * P : (j + 1) * P],
                            pxr[:].rearrange("p (s x) -> p s x", x=P),
                            op=OP.add,
                        )
                    z = wrk.tile([P, NBUCK * B1], BF16, tag="z")
                    nc.scalar.activation(
                        z[:], v[:], ACTF.Prelu, alpha=alpha_t[:, 0:1]
                    )
                    zv = z[:].rearrange("p (s x) -> p s x", x=D)
                    sc = small.tile([P, S], F32, tag="sc")
                    if l < 2:
                        # tables pre-scaled by |att| with positive features
                        # first: e = sum(z[:K]) - sum(z[K:])
                        K = Ks[l]
                        scn = small.tile([P, S], F32, tag="scn")
                        nc.vector.tensor_reduce(
                            sc[:], zv[:, :, 0:K], axis=AX.X, op=OP.add,
                        )
                        if K < D:
                            nc.vector.tensor_reduce(
                                scn[:], zv[:, :, K:D], axis=AX.X, op=OP.add,
                            )
                            nc.vector.tensor_tensor(
                                sc[:], sc[:], scn[:], op=OP.subtract
                            )
                    else:
                        t = wrk.tile([P, NBUCK * B1], BF16, tag="t")
                        nc.vector.tensor_tensor(
                            t[:].rearrange("p (s x) -> p s x", x=D),
                            zv,
                            at_t[l][:].unsqueeze(1).to_broadcast([P, S, D]),
                            op=OP.mult,
                        )
                        nc.vector.tensor_reduce(
                            sc[:], t[:].rearrange("p (s x) -> p s x", x=D),
                            axis=AX.X, op=OP.add,
                        )
                    w = small.tile([P, S], F32, tag="w")
                    nc.scalar.activation(w[:], sc[:], ACTF.Exp)
                    # one-hot (e-partition) for aggregation: OH[e, s*128+n]
                    oh = wrk.tile([P, S * P], BF16, tag="oh")
                    nc.vector.tensor_tensor(
                        oh[:].rearrange("p (s x) -> p s x", x=P),
                        iota_t[:].unsqueeze(1).to_broadcast([P, S, P]),
                        dlt[:, bl * S : (bl + 1) * S].unsqueeze(2).to_broadcast(
                            [P, S, P]
                        ),
                        op=OP.is_equal,
                    )
                    # rhs = [w*xl | w] (129 cols per slot column)
                    xlw = wrk.tile([P, S * 129], BF16, tag="xlw")
                    xlw_v = xlw[:].rearrange("p (s x) -> p s x", x=129)
                    nc.vector.tensor_tensor(
                        xlw_v[:, :, 0:P].rearrange(
                            "p (j k) x -> p j k x", k=NBUCK
                        ),
                        xlg_r[:, :, bl, :].rearrange(
                            "p k (j x) -> p j k x", x=P
                        ),
                        w[:].rearrange("p (j k) -> p j k", k=NBUCK)
                        .unsqueeze(3).to_broadcast([P, JJ, NBUCK, P]),
                        op=OP.mult,
                    )
                    nc.vector.tensor_scalar(
                        xlw_v[:, :, P : P + 1],
                        w[:].unsqueeze(2), 0.0, None, op0=OP.add,
                    )
                    pnum = psA.tile([P, 129], F32, tag="num")
                    for s in range(S):
                        nc.tensor.matmul(
                            pnum[:], oh[:, s * P : (s + 1) * P],
                            xlw_v[:, s, :],
                            start=(s == 0), stop=(s == S - 1),
                        )
                    den = small.tile([P, 1], F32, tag="den")
                    nc.vector.tensor_scalar(
                        den[:], pnum[:, P : P + 1], 1e-16, None, op0=OP.add
                    )
                    rec = small.tile([P, 1], F32, tag="rec")
                    nc.vector.reciprocal(rec[:], den[:])
                    onum = small.tile([P, D], F32, tag="onum")
                    nc.vector.tensor_scalar(
                        onum[:], pnum[:, 0:P], rec[:], None, op0=OP.mult
                    )
                    nc.vector.tensor_tensor(onum[:], onum[:], bi_t[l][:], op=OP.add)
                    if l == 2:
                        nc.sync.dma_start(out_loc[b * P : b * P + bw, :], onum[:bw, :])
                    else:
                        hrow = small.tile([P, D], BF16, tag="hrow")
                        nc.scalar.activation(hrow[:], onum[:], ACTF.Relu)
                        pst = psT.tile([P, P], BF16, tag="fint")
                        nc.tensor.transpose(pst[:], hrow[:], ident_t[:])
                        hT = small.tile([P, P], BF16, tag="hT")
                        nc.scalar.activation(hT[:], pst[:], ACTF.Copy)
                        pxl = psF.tile([P, D], F32, tag="fin")
                        nc.tensor.matmul(
                            pxl[:], hT[:], wl_t[l + 1][:], start=True, stop=True
                        )
                        sxl = small.tile([P, D], BF16, tag="sxl")
                        nc.scalar.activation(sxl[:], pxl[:], ACTF.Copy)
                        nc.sync.dma_start(
                            XLb[l + 1][b * P : b * P + bw, :], sxl[:bw, :]
                        )
                        pxr2 = psF.tile([P, D], F32, tag="fin")
                        nc.tensor.matmul(
                            pxr2[:], hT[:], wr_t[l + 1][:], start=True, stop=True
                        )
                        nc.scalar.activation(
                            XR_sb[:, b * D : (b + 1) * D], pxr2[:], ACTF.Copy
                        )
                goff += NBUCK * gcols
            if l < 2:
                nc.gpsimd.collective_compute(
                    "AllGather", OP.bypass,
                    replica_groups=[list(range(cfg.CORES))],
                    ins=[XLb[l + 1].ap().opt()], outs=[XLf[l + 1].ap().opt()],
                )
    nc.compile()
    return nc


def kernel(x, Wl, Wr, att, b, edge_index):
    x = np.asarray(x, np.float32)
    edge_index = np.asarray(edge_index)
    N = x.shape[0]
    CORES = 8

    # uniform slot budget from this input's worst (core, block, bucket)
    bucket = cdiv(N, 4)
    src = np.asarray(edge_index[0], np.int64)
    dst = np.asarray(edge_index[1], np.int64)
    npc = N // CORES
    nblk = cdiv(npc, P)
    mx = 0
    for c in range(CORES):
        m = (dst >= c * npc) & (dst < (c + 1) * npc)
        key = ((dst[m] - c * npc) // P) * 4 + src[m] // bucket
        mx = max(mx, int(np.bincount(key, minlength=nblk * 4).max()))
    b1 = max(cdiv(mx, P) * P, P)

    cfg = Cfg(N=N, cores=CORES, bucket=bucket, b1=b1, sb=4)
    idx_data = host_prep(cfg, edge_index)
    const_data = host_consts(cfg, Wl, Wr, att, b, x)
    att_np = np.asarray(att, np.float32)
    Ks = [int((att_np[l] >= 0).sum()) for l in range(2)]
    nc = build_program(cfg, Ks)
    in_maps = [{**idx_data[c], **const_data[c]} for c in range(CORES)]

    prof_dir = os.environ.get("GAT_PROFILE", "")
    if prof_dir:
        import sys
        sys.path.insert(0, "/root/.axon_site")
        from trn_agent_boot import trn_boot
        hook = trn_boot._ntff_profile_via_ctypes("/opt/axon/libaxon_pjrt.so")
        os.makedirs(prof_dir, exist_ok=True)
        with hook(prof_dir, [0]):
            res = run_bass_kernel_spmd(nc, in_maps, core_ids=list(range(CORES)))
    else:
        res = run_bass_kernel_spmd(nc, in_maps, core_ids=list(range(CORES)))

    out = np.concatenate([r["out_loc"] for r in res.results], axis=0)
    return out.astype(np.float32)


# revision 18
# speedup vs baseline: 2.9109x; 2.9109x over previous
"""3-layer GATv2 (heads=1, eval) on 8 Trainium2 NeuronCores — Bass/Tile.

kernel(**inputs) takes the FULL inputs (x [100000,128] f32, Wl/Wr [3,128,128],
att [3,128], b [3,128], edge_index [2,1600000] int64) and returns the FULL
[100000, 128] float32 output.

Strategy (graph/data parallel, node-partitioned):
  * core c owns dst nodes [c*12500, (c+1)*12500); edges grouped on the host by
    (dst block of 128 nodes, src bucket of 25000 rows) with a uniform
    per-(block,bucket) slot budget B1 (multiple of 128). Slot order within a
    block is j-major: s = j*NBUCK + k (j = 128-slot sub-column, k = bucket) so
    one 512-slot chunk = one j across all 4 buckets.
  * per layer, XL = h@Wl for all N rows (bf16) lives in DRAM (built locally,
    AllGather'd); XR = h@Wr for local rows lives in a persistent SBUF tile.
  * ONLY xl is fetched per-edge, with SWDGE dma_gather (int16 idx, 256B rows)
    spread over 4 SWDGE queues (one per bucket). xr per edge is expanded on
    the PE: one-hot OHsT[n, e] = [dloc_e == n] (one batched DVE is_equal from
    a host-replicated int8 dloc table) times the 128-row XR block.
  * scores: v = xl[src]+xr[dst]; z = LeakyReLU(v) on the scalar engine;
    e = sum_d z*att (DVE mult+reduce); w = exp(e) (no segment-max: |e| < ~30).
  * aggregation per block on PE: lhsT = OH[e, n] (batched DVE is_equal build),
    rhs = [w*xl | w] (129 cols) accumulated over the 20 slot columns in PSUM;
    col 128 gives the softmax denominator. out = nums/den + bias (+relu).
  * next layer's XL rows + XR block produced in the same block pass
    (PE transpose + two 128x128 matmuls); only the AllGather separates layers.
"""

import os
from contextlib import ExitStack

import numpy as np
import ml_dtypes

import concourse.bacc as bacc
import concourse.mybir as mybir
import concourse.tile as tile
from concourse._compat import cdiv
from concourse.masks import make_identity
from concourse.bass_utils import run_bass_kernel_spmd

F32 = mybir.dt.float32
BF16 = mybir.dt.bfloat16
I16 = mybir.dt.int16
I8 = mybir.dt.int8
AX = mybir.AxisListType
OP = mybir.AluOpType
ACTF = mybir.ActivationFunctionType

D = 128
P = 128


class Cfg:
    def __init__(self, N, cores, bucket, b1, sb):
        assert N % cores == 0
        self.N, self.CORES = N, cores
        self.NPC = N // cores
        self.NBLK = cdiv(self.NPC, P)
        self.LASTW = self.NPC - (self.NBLK - 1) * P
        self.BUCKET = bucket
        self.NBUCK = cdiv(N, bucket)
        assert b1 % P == 0
        self.B1 = b1
        self.JJ = b1 // P
        self.S = self.NBUCK * self.JJ
        self.SB = sb
        self.NSB = cdiv(self.NBLK, sb)
        self.IDXCOLS_TOT = sum(
            self.sbn(g) * self.B1 // 16 * self.NBUCK for g in range(self.NSB)
        )

    def sbn(self, g):
        return min(self.SB, self.NBLK - g * self.SB)


def _wrap16(v):
    L = v.size
    assert L % 16 == 0
    w = v.reshape(L // 16, 16).T.astype(np.int16)
    return np.tile(w, (8, 1))


def host_prep(cfg, edge_index):
    src = np.asarray(edge_index[0], dtype=np.int64)
    dst = np.asarray(edge_index[1], dtype=np.int64)
    cores = []
    for c in range(cfg.CORES):
        base = c * cfg.NPC
        m = (dst >= base) & (dst < base + cfg.NPC)
        es, ed = src[m], dst[m] - base
        blk = ed // P
        buck = es // cfg.BUCKET
        order = np.lexsort((es, buck, blk))
        es, ed, blk, buck = es[order], ed[order], blk[order], buck[order]
        key = blk * cfg.NBUCK + buck
        bounds = np.searchsorted(key, np.arange(cfg.NBLK * cfg.NBUCK + 1))
        cnt = np.diff(bounds).reshape(cfg.NBLK, cfg.NBUCK)
        if cnt.max() > cfg.B1:
            raise ValueError(f"bucket overflow: {cnt.max()} > {cfg.B1}")
        xl_slots = np.zeros((cfg.NBLK, cfg.NBUCK, cfg.B1), np.int64)
        dl_slots = np.full((cfg.NBLK, cfg.NBUCK, cfg.B1), -1.0, np.float32)
        for b in range(cfg.NBLK):
            for k in range(cfg.NBUCK):
                i0, i1 = bounds[b * cfg.NBUCK + k], bounds[b * cfg.NBUCK + k + 1]
                n = i1 - i0
                xl_slots[b, k, :n] = es[i0:i1] - k * cfg.BUCKET
                dl_slots[b, k, :n] = (ed[i0:i1] - b * P).astype(np.float32)
        xl_cols = []
        for g in range(cfg.NSB):
            sbn = cfg.sbn(g)
            for k in range(cfg.NBUCK):
                xl_cols.append(
                    _wrap16(xl_slots[g * cfg.SB : g * cfg.SB + sbn, k, :].reshape(-1))
                )
        xl_idx = np.concatenate(xl_cols, axis=1)
        # j-major slot order: s = j*NBUCK + k
        dl = (
            dl_slots.reshape(cfg.NBLK, cfg.NBUCK, cfg.JJ, P)
            .transpose(0, 2, 1, 3)
            .reshape(cfg.NBLK, cfg.S, P)
        )
        dloc = np.ascontiguousarray(
            dl.transpose(2, 0, 1).reshape(P, cfg.NBLK * cfg.S)
        ).astype(ml_dtypes.bfloat16)
        dlb_row = dl.reshape(cfg.NBLK * cfg.S * P).astype(np.int8)
        dlb = np.ascontiguousarray(
            np.broadcast_to(dlb_row[None, :], (P, dlb_row.size))
        )
        cores.append(dict(xl_idx=xl_idx, dloc=dloc, dlb=dlb))
    return cores


def host_consts(cfg, Wl, Wr, att, b, x):
    Wl = np.asarray(Wl, np.float32).copy()
    Wr = np.asarray(Wr, np.float32).copy()
    att = np.asarray(att, np.float32)
    b = np.asarray(b, np.float32).copy()
    x = np.asarray(x, np.float32)
    # fold |att| + sign-sorted feature permutation into layers 0/1 so the
    # per-edge score is a sign-split column sum; layer 2 stays plain.
    perms, scales, Ks = [], [], []
    for l in range(2):
        a = att[l]
        pos = np.where(a >= 0)[0]
        neg = np.where(a < 0)[0]
        perm = np.concatenate([pos, neg])
        s = np.maximum(np.abs(a[perm]), 1e-6)
        perms.append(perm); scales.append(s); Ks.append(len(pos))
    Wl_e = [None] * 3
    Wr_e = [None] * 3
    b_e = [None] * 3
    Wl_e[0] = Wl[0][:, perms[0]] * scales[0][None, :]
    Wr_e[0] = Wr[0][:, perms[0]] * scales[0][None, :]
    b_e[0] = b[0][perms[0]] * scales[0]
    Wl_e[1] = ((1.0 / scales[0])[:, None] * Wl[1][perms[0], :])[:, perms[1]] \
        * scales[1][None, :]
    Wr_e[1] = ((1.0 / scales[0])[:, None] * Wr[1][perms[0], :])[:, perms[1]] \
        * scales[1][None, :]
    b_e[1] = b[1][perms[1]] * scales[1]
    Wl_e[2] = (1.0 / scales[1])[:, None] * Wl[2][perms[1], :]
    Wr_e[2] = (1.0 / scales[1])[:, None] * Wr[2][perms[1], :]
    b_e[2] = b[2]
    wl_all = np.stack(Wl_e).reshape(3 * D, D).astype(ml_dtypes.bfloat16)
    wr_all = np.stack(Wr_e).reshape(3 * D, D).astype(ml_dtypes.bfloat16)
    att_mat = np.concatenate(
        [np.tile(att[l][None, :], (P, 1)) for l in range(3)], 0
    ).astype(ml_dtypes.bfloat16)
    bias_mat = np.concatenate(
        [np.tile(b_e[l][None, :], (P, 1)) for l in range(3)], 0
    ).astype(np.float32)
    iota = np.tile(np.arange(P, dtype=np.float32)[None, :], (P, 1)).astype(
        ml_dtypes.bfloat16
    )
    iota_col = np.arange(P, dtype=np.float32).reshape(P, 1)
    out = []
    for c in range(cfg.CORES):
        xT = np.ascontiguousarray(x[c * cfg.NPC : (c + 1) * cfg.NPC].T).astype(
            ml_dtypes.bfloat16
        )
        out.append(
            dict(
                xT_loc=xT,
                Wl_all=wl_all,
                Wr_all=wr_all,
                att_mat=att_mat,
                bias_mat=bias_mat,
                iota_mat=iota,
                iota_col=iota_col,
            )
        )
    return out


def build_program(cfg, Ks):
    nc = bacc.Bacc(
        "TRN2", target_bir_lowering=False, debug=False, num_devices=cfg.CORES,
        num_swdge_queues=4,
    )
    NPC, NBLK, NBUCK, B1, S, SB, NSB, JJ = (
        cfg.NPC, cfg.NBLK, cfg.NBUCK, cfg.B1, cfg.S, cfg.SB, cfg.NSB, cfg.JJ,
    )

    xT_loc = nc.dram_tensor("xT_loc", [P, NPC], BF16, kind="ExternalInput")
    Wl_all = nc.dram_tensor("Wl_all", [3 * D, D], BF16, kind="ExternalInput")
    Wr_all = nc.dram_tensor("Wr_all", [3 * D, D], BF16, kind="ExternalInput")
    att_mat = nc.dram_tensor("att_mat", [3 * P, D], BF16, kind="ExternalInput")
    bias_mat = nc.dram_tensor("bias_mat", [3 * P, D], F32, kind="ExternalInput")
    iota_mat = nc.dram_tensor("iota_mat", [P, P], BF16, kind="ExternalInput")
    iota_col = nc.dram_tensor("iota_col", [P, 1], F32, kind="ExternalInput")
    xl_idx = nc.dram_tensor("xl_idx", [P, cfg.IDXCOLS_TOT], I16, kind="ExternalInput")
    dloc = nc.dram_tensor("dloc", [P, NBLK * S], BF16, kind="ExternalInput")
    dlb_d = nc.dram_tensor("dlb", [P, NBLK * S * P], I8, kind="ExternalInput")
    out_loc = nc.dram_tensor("out_loc", [NPC, D], F32, kind="ExternalOutput")

    XLb = [nc.dram_tensor(f"XLb{l}", [NPC, D], BF16) for l in range(3)]
    XLf = [nc.dram_tensor(f"XLf{l}", [cfg.N, D], BF16) for l in range(3)]

    with tile.TileContext(nc) as tc, ExitStack() as ctx:
        consts = ctx.enter_context(tc.tile_pool(name="consts", bufs=1))
        gpool = ctx.enter_context(tc.tile_pool(name="gath", bufs=2))
        wrk = ctx.enter_context(tc.tile_pool(name="wrk", bufs=2))
        small = ctx.enter_context(tc.tile_pool(name="small", bufs=3))
        idxp = ctx.enter_context(tc.tile_pool(name="idx", bufs=2))
        dlbp = ctx.enter_context(tc.tile_pool(name="dlb", bufs=2))
        psX = ctx.enter_context(tc.tile_pool(name="psX", bufs=2, space="PSUM"))
        psA = ctx.enter_context(tc.tile_pool(name="psA", bufs=2, space="PSUM"))
        psF = ctx.enter_context(tc.tile_pool(name="psF", bufs=2, space="PSUM"))
        psT = ctx.enter_context(tc.tile_pool(name="psT", bufs=2, space="PSUM"))

        iota_t = consts.tile([P, P], BF16, tag="iota")
        nc.sync.dma_start(iota_t[:], iota_mat[:, :])
        iotac = consts.tile([P, 1], F32, tag="iotac")
        nc.sync.dma_start(iotac[:], iota_col[:, :])
        alpha_t = consts.tile([P, 1], F32, tag="alpha")
        nc.vector.memset(alpha_t[:], 0.2)
        ident_t = consts.tile([P, P], BF16, tag="ident")
        make_identity(nc, ident_t[:])
        XR_sb = consts.tile([P, NBLK * D], BF16, tag="xrsb")
        wl_t, wr_t, at_t, bi_t = [], [], [], []
        for l in range(3):
            w1 = consts.tile([P, D], BF16, tag=f"wl{l}")
            nc.sync.dma_start(w1[:], Wl_all[l * D : (l + 1) * D, :])
            w2 = consts.tile([P, D], BF16, tag=f"wr{l}")
            nc.sync.dma_start(w2[:], Wr_all[l * D : (l + 1) * D, :])
            a1 = consts.tile([P, D], BF16, tag=f"att{l}")
            nc.sync.dma_start(a1[:], att_mat[l * P : (l + 1) * P, :])
            b1t = consts.tile([P, D], F32, tag=f"bias{l}")
            nc.sync.dma_start(b1t[:], bias_mat[l * P : (l + 1) * P, :])
            wl_t.append(w1); wr_t.append(w2); at_t.append(a1); bi_t.append(b1t)

        for cblk in range(NBLK):
            cw = P if cblk < NBLK - 1 else cfg.LASTW
            xTs = wrk.tile([P, P], BF16, tag="xTs")
            if cw < P:
                nc.vector.memset(xTs[:, cw:], 0.0)
            nc.sync.dma_start(xTs[:, :cw], xT_loc[:, cblk * P : cblk * P + cw])
            pxl = psF.tile([P, D], F32, tag="fin")
            nc.tensor.matmul(pxl[:], xTs[:], wl_t[0][:], start=True, stop=True)
            sxl = small.tile([P, D], BF16, tag="sxl")
            nc.scalar.activation(sxl[:cw, :], pxl[:cw, :], ACTF.Copy)
            nc.sync.dma_start(XLb[0][cblk * P : cblk * P + cw, :], sxl[:cw, :])
            pxr = psF.tile([P, D], F32, tag="fin")
            nc.tensor.matmul(pxr[:], xTs[:], wr_t[0][:], start=True, stop=True)
            nc.scalar.activation(
                XR_sb[:, cblk * D : cblk * D + D], pxr[:], ACTF.Copy
            )
        nc.gpsimd.collective_compute(
            "AllGather", OP.bypass,
            replica_groups=[list(range(cfg.CORES))],
            ins=[XLb[0].ap().opt()], outs=[XLf[0].ap().opt()],
        )

        for l in range(3):
            goff = 0
            for g in range(NSB):
                sbn = cfg.sbn(g)
                gcols = sbn * B1 // 16
                ixl = idxp.tile([P, NBUCK * gcols], I16, tag="ixl")
                nc.sync.dma_start(ixl[:], xl_idx[:, goff : goff + NBUCK * gcols])
                dlt = small.tile([P, sbn * S], BF16, tag="dlt")
                nc.sync.dma_start(
                    dlt[:], dloc[:, g * SB * S : g * SB * S + sbn * S]
                )
                dlb = dlbp.tile([P, sbn * S * P], I8, tag="dlb")
                nc.sync.dma_start(
                    dlb[:],
                    dlb_d[:, g * SB * S * P : g * SB * S * P + sbn * S * P],
                )
                xlg = gpool.tile([P, NBUCK * sbn * B1], BF16, tag="xlg")
                ni = sbn * B1
                for k in range(NBUCK):
                    kb = k * cfg.BUCKET
                    ke = min(kb + cfg.BUCKET, cfg.N)
                    nc.gpsimd.dma_gather(
                        xlg[:, k * ni : (k + 1) * ni].rearrange("p (m x) -> p m x", x=D),
                        XLf[l][kb:ke, :],
                        ixl[:, k * gcols : (k + 1) * gcols],
                        ni, ni, D, single_packet=False,
                        queue_num=k,
                    )
                # [P, NBUCK, sbn, JJ*128] per-bucket rows of this group
                xlg_r = xlg[:].rearrange(
                    "p (k s b1) -> p k s b1", k=NBUCK, s=sbn
                )
                for bl in range(sbn):
                    b = g * SB + bl
                    bw = P if b < NBLK - 1 else cfg.LASTW
                    # one-hot (n-partition) for xr expansion: OHsT[n, s*128+e]
                    ohsT = wrk.tile([P, S * P], BF16, tag="ohsT")
                    nc.vector.tensor_scalar(
                        ohsT[:],
                        dlb[:, bl * S * P : (bl + 1) * S * P],
                        iotac[:, 0:1], None, op0=OP.is_equal,
                    )
                    xrblk = XR_sb[:, b * D : (b + 1) * D]
                    v = wrk.tile([P, NBUCK * B1], BF16, tag="v")
                    for j in range(JJ):
                        pxr = psX.tile([P, NBUCK * P], F32, tag="px")
                        for k in range(NBUCK):
                            s = j * NBUCK + k
                            nc.tensor.matmul(
                                pxr[:, k * P : (k + 1) * P],
                                ohsT[:, s * P : (s + 1) * P],
                                xrblk,
                                start=True, stop=True,
                            )
                        nc.vector.tensor_tensor(
                            v[:].rearrange("p (s x) -> p s x", x=P)[
                                :, j * NBUCK : (j + 1) * NBUCK, :
                            ],
                            xlg_r[:, :, bl, j * P : (j + 1) * P],
                            pxr[:].rearrange("p (s x) -> p s x", x=P),
                            op=OP.add,
                        )
                    z = wrk.tile([P, NBUCK * B1], BF16, tag="z")
                    nc.scalar.activation(
                        z[:], v[:], ACTF.Prelu, alpha=alpha_t[:, 0:1]
                    )
                    zv = z[:].rearrange("p (s x) -> p s x", x=D)
                    sc = small.tile([P, S], F32, tag="sc")
                    if l < 2:
                        # tables pre-scaled by |att| with positive features
                        # first: e = sum(z[:K]) - sum(z[K:])
                        K = Ks[l]
                        scn = small.tile([P, S], F32, tag="scn")
                        nc.vector.tensor_reduce(
                            sc[:], zv[:, :, 0:K], axis=AX.X, op=OP.add,
                        )
                        if K < D:
                            nc.vector.tensor_reduce(
                                scn[:], zv[:, :, K:D], axis=AX.X, op=OP.add,
                            )
                            nc.vector.tensor_tensor(
                                sc[:], sc[:], scn[:], op=OP.subtract
                            )
                    else:
                        t = wrk.tile([P, NBUCK * B1], BF16, tag="t")
                        nc.vector.tensor_tensor(
                            t[:].rearrange("p (s x) -> p s x", x=D),
                            zv,
                            at_t[l][:].unsqueeze(1).to_broadcast([P, S, D]),
                            op=OP.mult,
                        )
                        nc.vector.tensor_reduce(
                            sc[:], t[:].rearrange("p (s x) -> p s x", x=D),
                            axis=AX.X, op=OP.add,
                        )
                    w = small.tile([P, S], BF16, tag="w")
                    nc.scalar.activation(w[:], sc[:], ACTF.Exp)
                    # one-hot (e-partition) for aggregation: OH[e, s*128+n]
                    oh = wrk.tile([P, S * P], BF16, tag="oh")
                    nc.vector.tensor_tensor(
                        oh[:].rearrange("p (s x) -> p s x", x=P),
                        iota_t[:].unsqueeze(1).to_broadcast([P, S, P]),
                        dlt[:, bl * S : (bl + 1) * S].unsqueeze(2).to_broadcast(
                            [P, S, P]
                        ),
                        op=OP.is_equal,
                    )
                    # rhs = [w*xl | w] (129 cols per slot column)
                    xlw = wrk.tile([P, S * 129], BF16, tag="xlw")
                    xlw_v = xlw[:].rearrange("p (s x) -> p s x", x=129)
                    nc.vector.tensor_tensor(
                        xlw_v[:, :, 0:P].rearrange(
                            "p (j k) x -> p j k x", k=NBUCK
                        ),
                        xlg_r[:, :, bl, :].rearrange(
                            "p k (j x) -> p j k x", x=P
                        ),
                        w[:].rearrange("p (j k) -> p j k", k=NBUCK)
                        .unsqueeze(3).to_broadcast([P, JJ, NBUCK, P]),
                        op=OP.mult,
                    )
                    nc.vector.tensor_scalar(
                        xlw_v[:, :, P : P + 1],
                        w[:].unsqueeze(2), 0.0, None, op0=OP.add,
                    )
                    pnum = psA.tile([P, 129], F32, tag="num")
                    for s in range(S):
                        nc.tensor.matmul(
                            pnum[:], oh[:, s * P : (s + 1) * P],
                            xlw_v[:, s, :],
                            start=(s == 0), stop=(s == S - 1),
                        )
                    den = small.tile([P, 1], F32, tag="den")
                    nc.vector.tensor_scalar(
                        den[:], pnum[:, P : P + 1], 1e-16, None, op0=OP.add
                    )
                    rec = small.tile([P, 1], F32, tag="rec")
                    nc.vector.reciprocal(rec[:], den[:])
                    onum = small.tile([P, D], F32, tag="onum")
                    nc.vector.tensor_scalar(
                        onum[:], pnum[:, 0:P], rec[:], None, op0=OP.mult
                    )
                    nc.vector.tensor_tensor(onum[:], onum[:], bi_t[l][:], op=OP.add)
                    if l == 2:
                        nc.sync.dma_start(out_loc[b * P : b * P + bw, :], onum[:bw, :])
                    else:
                        hrow = small.tile([P, D], BF16, tag="hrow")
                        nc.scalar.activation(hrow[:], onum[:], ACTF.Relu)
                        pst = psT.tile([P, P], BF16, tag="fint")
                        nc.tensor.transpose(pst[:], hrow[:], ident_t[:])
                        hT = small.tile([P, P], BF16, tag="hT")
                        nc.scalar.activation(hT[:], pst[:], ACTF.Copy)
                        pxl = psF.tile([P, D], F32, tag="fin")
                        nc.tensor.matmul(
                            pxl[:], hT[:], wl_t[l + 1][:], start=True, stop=True
                        )
                        sxl = small.tile([P, D], BF16, tag="sxl")
                        nc.scalar.activation(sxl[:], pxl[:], ACTF.Copy)
                        nc.sync.dma_start(
                            XLb[l + 1][b * P : b * P + bw, :], sxl[:bw, :]
                        )
                        pxr2 = psF.tile([P, D], F32, tag="fin")
                        nc.tensor.matmul(
                            pxr2[:], hT[:], wr_t[l + 1][:], start=True, stop=True
                        )
                        nc.scalar.activation(
                            XR_sb[:, b * D : (b + 1) * D], pxr2[:], ACTF.Copy
                        )
                goff += NBUCK * gcols
            if l < 2:
                nc.gpsimd.collective_compute(
                    "AllGather", OP.bypass,
                    replica_groups=[list(range(cfg.CORES))],
                    ins=[XLb[l + 1].ap().opt()], outs=[XLf[l + 1].ap().opt()],
                )
    nc.compile()
    return nc


def kernel(x, Wl, Wr, att, b, edge_index):
    x = np.asarray(x, np.float32)
    edge_index = np.asarray(edge_index)
    N = x.shape[0]
    CORES = 8

    # uniform slot budget from this input's worst (core, block, bucket)
    bucket = cdiv(N, 4)
    src = np.asarray(edge_index[0], np.int64)
    dst = np.asarray(edge_index[1], np.int64)
    npc = N // CORES
    nblk = cdiv(npc, P)
    mx = 0
    for c in range(CORES):
        m = (dst >= c * npc) & (dst < (c + 1) * npc)
        key = ((dst[m] - c * npc) // P) * 4 + src[m] // bucket
        mx = max(mx, int(np.bincount(key, minlength=nblk * 4).max()))
    b1 = max(cdiv(mx, P) * P, P)

    cfg = Cfg(N=N, cores=CORES, bucket=bucket, b1=b1, sb=4)
    idx_data = host_prep(cfg, edge_index)
    const_data = host_consts(cfg, Wl, Wr, att, b, x)
    att_np = np.asarray(att, np.float32)
    Ks = [int((att_np[l] >= 0).sum()) for l in range(2)]
    nc = build_program(cfg, Ks)
    in_maps = [{**idx_data[c], **const_data[c]} for c in range(CORES)]

    prof_dir = os.environ.get("GAT_PROFILE", "")
    if prof_dir:
        import sys
        sys.path.insert(0, "/root/.axon_site")
        from trn_agent_boot import trn_boot
        hook = trn_boot._ntff_profile_via_ctypes("/opt/axon/libaxon_pjrt.so")
        os.makedirs(prof_dir, exist_ok=True)
        with hook(prof_dir, [0]):
            res = run_bass_kernel_spmd(nc, in_maps, core_ids=list(range(CORES)))
    else:
        res = run_bass_kernel_spmd(nc, in_maps, core_ids=list(range(CORES)))

    out = np.concatenate([r["out_loc"] for r in res.results], axis=0)
    return out.astype(np.float32)
